# revision 6
# baseline (speedup 1.0000x reference)
"""Collaborative multi-head attention kernel for Trainium2 (8 NeuronCores).

Problem (hardcoded shapes): x:(S=1024, B=8, E=1024) fp32, 16 heads, Dh=64.
    q = (x @ w_q) * Dh**-0.5 ; k = x @ w_k ; v = x + x @ w_v
    per (b, h): attn = softmax(q_h k_h^T + mask[b]) ; ctx = attn @ v_h
    out = ctx @ w_out + b_out

Sharding: pure data parallel over batch — core c computes batch c entirely.
No collectives needed.

Per-core kernel design (bf16 matmuls, fp32 accumulation / softmax):
  - x is loaded [s,e], cast to bf16, transposed on PE -> xT [e,s].
  - QT = (x w_q)^T and KT = (x w_k)^T computed directly in [e', s] layout
    (lhsT = w tile, rhs = xT). V natural [s, e'] with fused +x residual.
  - Scores are computed TRANSPOSED, scT[s, t] per head, so that
    * the additive key mask is a per-partition bias fused into the exp
      activation (out = Exp(0.125 * in + mask[s])),
    * softmax denominators are summed over s by an all-ones stationary
      matmul, and
    * the exp output attnT[s, t] is directly the rhs of the ctx matmul.
  - Head pairs share the PE array: QK^T row-tiled (K=64 at rows 0/64),
    ctx col-tiled (h0 -> psum rows 0:64, h1 -> rows 64:128), denominator
    sums pre-broadcast over 64 rows by an all-ones [128, 64] stationary.
  - Softmax skips max-subtraction: |scores| <= ~20 for these inputs, far
    inside fp32 exp range.
  - Normalization (1/sum) is fused into the PSUM->SBUF evacuation of ctx.
  - out = ctxT^T @ w_out accumulated in PSUM; b_out added by a K=1 matmul
    of ones^T @ b_out into the same accumulation group.
"""

import numpy as np

from concourse import bass, mybir, tile, bacc
from concourse.bass_utils import run_bass_kernel_spmd
from concourse.masks import make_identity

F32 = mybir.dt.float32
BF16 = mybir.dt.bfloat16

S = 1024
E = 1024
H = 16
DH = 64
N_CORES = 8
P = 128
NB = S // P          # 8 s-blocks / e-tiles / t-tiles
HP = H // 2          # 8 head pairs
SCALE = DH ** -0.5   # 0.125
NT = 512             # matmul moving free dim (one PSUM bank fp32)


def build_kernel(reps: int = 1):
    """Build + bass-compile the SPMD program (same program for all 8 cores).

    reps > 1 wraps the whole body in a dynamic loop (used only by the local
    timing harness; the graded path uses reps=1).
    """
    nc = bacc.Bacc("TRN2", target_bir_lowering=False, debug=False,
                   num_devices=N_CORES)

    x_d = nc.dram_tensor("x", [S, E], F32, kind="ExternalInput")
    mask_d = nc.dram_tensor("mask", [S], F32, kind="ExternalInput")
    wq_d = nc.dram_tensor("w_q", [E, E], F32, kind="ExternalInput")
    wk_d = nc.dram_tensor("w_k", [E, E], F32, kind="ExternalInput")
    wv_d = nc.dram_tensor("w_v", [E, E], F32, kind="ExternalInput")
    wo_d = nc.dram_tensor("w_out", [E, E], F32, kind="ExternalInput")
    bo_d = nc.dram_tensor("b_out", [E], F32, kind="ExternalInput")
    out_d = nc.dram_tensor("out", [S, E], F32, kind="ExternalOutput")

    with tile.TileContext(nc) as tc:
        if reps == 1:
            _emit_body(tc, nc, x_d, mask_d, wq_d, wk_d, wv_d, wo_d, bo_d, out_d)
        else:
            with tc.For_i(0, reps, 1):
                _emit_body(tc, nc, x_d, mask_d, wq_d, wk_d, wv_d, wo_d, bo_d,
                           out_d)

    nc.compile()
    return nc


def _emit_body(tc, nc, x_d, mask_d, wq_d, wk_d, wv_d, wo_d, bo_d, out_d):
    with tc.tile_pool(name="consts", bufs=1) as consts, \
         tc.tile_pool(name="persist", bufs=1) as persist, \
         tc.tile_pool(name="stage", bufs=2) as stage:

        # ---- constants ----
        ident = consts.tile([P, P], BF16)
        make_identity(nc, ident[:])
        ones64 = consts.tile([P, DH], BF16)
        nc.gpsimd.memset(ones64[:], 1.0)
        ones_k1 = consts.tile([1, P], F32)
        nc.gpsimd.memset(ones_k1[:], 1.0)
        mask_t = consts.tile([P, NB], F32)
        nc.sync.dma_start(mask_t[:], mask_d.ap().rearrange("(a b) -> b a", b=P))
        bo_sb = consts.tile([1, E], F32)
        nc.sync.dma_start(bo_sb[:], bo_d.ap().rearrange("(a b) -> a b", a=1))

        # ---- persistent bf16 tensors ----
        x_bf = persist.tile([P, NB, E], BF16)     # x natural; [:, sb, e]
        xT = persist.tile([P, NB, S], BF16)       # xT; [:, k, s], e = k*128+p
        qt = persist.tile([P, NB, S], BF16)       # QT; [:, i, t], e' = i*128+p
        kt = persist.tile([P, NB, S], BF16)       # KT
        v_sb = persist.tile([P, NB, E], BF16)     # V natural; [:, sb, e']
        ctxT = persist.tile([P, NB, S], BF16)     # normalized ctx^T; [:, hp, t]
        wo_bf = persist.tile([P, NB, E], BF16)    # w_out bf16; [:, k, e'']

        # ---- load x, cast to bf16 ----
        for sb in range(NB):
            xs = stage.tile([P, E], F32, tag="xs")
            nc.sync.dma_start(xs[:], x_d.ap()[sb * P:(sb + 1) * P, :])
            nc.vector.tensor_copy(x_bf[:, sb, :], xs[:])

        def load_weight_bf16(w_d, dest):
            for k in range(NB):
                ws = stage.tile([P, E], F32, tag="ws")
                nc.sync.dma_start(ws[:], w_d.ap()[k * P:(k + 1) * P, :])
                nc.vector.tensor_copy(dest[:, k, :], ws[:])

        w_bf = persist.tile([P, NB, E], BF16, tag="w_bf")    # q then v
        w_bf2 = persist.tile([P, NB, E], BF16, tag="w_bf2")  # k

        # ---- transpose x on PE: xT[:, k, sb*128:...] = x_bf[:, sb, k*128:..]^T
        with tc.tile_pool(name="tp_psum", bufs=2, space="PSUM") as tp_pool:
            for k in range(NB):
                for sb in range(NB):
                    tp = tp_pool.tile([P, P], BF16, tag="tp")
                    nc.tensor.transpose(tp[:], x_bf[:, sb, k * P:(k + 1) * P],
                                        ident[:])
                    nc.vector.tensor_copy(xT[:, k, sb * P:(sb + 1) * P], tp[:])

        # ---- projections ----
        with tc.tile_pool(name="proj_psum", bufs=2, space="PSUM") as proj_psum:
            # QT: out[e', s] = sum_e w_q[e, e'] xT[e, s]
            load_weight_bf16(wq_d, w_bf)
            for i in range(NB):
                ps = proj_psum.tile([P, S], F32, tag="proj")
                for k in range(NB):
                    for c in range(S // NT):
                        nc.tensor.matmul(
                            ps[:, c * NT:(c + 1) * NT],
                            w_bf[:, k, i * P:(i + 1) * P],
                            xT[:, k, c * NT:(c + 1) * NT],
                            start=(k == 0), stop=(k == NB - 1))
                nc.scalar.copy(qt[:, i, :], ps[:])

            load_weight_bf16(wk_d, w_bf2)
            for i in range(NB):
                ps = proj_psum.tile([P, S], F32, tag="proj")
                for k in range(NB):
                    for c in range(S // NT):
                        nc.tensor.matmul(
                            ps[:, c * NT:(c + 1) * NT],
                            w_bf2[:, k, i * P:(i + 1) * P],
                            xT[:, k, c * NT:(c + 1) * NT],
                            start=(k == 0), stop=(k == NB - 1))
                nc.scalar.copy(kt[:, i, :], ps[:])

            # V: out[s, e'] = x[s, e'] + sum_e xT[e, s] w_v[e, e']
            load_weight_bf16(wv_d, w_bf)
            for sb in range(NB):
                ps = proj_psum.tile([P, S], F32, tag="proj")
                for k in range(NB):
                    for c in range(S // NT):
                        nc.tensor.matmul(
                            ps[:, c * NT:(c + 1) * NT],
                            xT[:, k, sb * P:(sb + 1) * P],
                            w_bf[:, k, c * NT:(c + 1) * NT],
                            start=(k == 0), stop=(k == NB - 1))
                nc.vector.tensor_add(v_sb[:, sb, :], ps[:], x_bf[:, sb, :])

            load_weight_bf16(wo_d, wo_bf)

        # ---- attention ----
        with tc.tile_pool(name="sc_psum", bufs=2, space="PSUM") as sc_pool, \
             tc.tile_pool(name="ab_psum", bufs=1, space="PSUM") as ab_pool, \
             tc.tile_pool(name="attn", bufs=3) as attn_pool, \
             tc.tile_pool(name="recip", bufs=2) as recip_pool:

            for hp in range(HP):
                h0, h1 = 2 * hp, 2 * hp + 1
                for th in range(S // NT):
                    t0 = th * NT
                    # One PSUM accumulation group per bank (hardware zero-
                    # region rule): ctx h0 / ctx h1 / sums h0 / sums h1 each
                    # get their own bank. h1 results live at psum rows
                    # 64:128 (col-tiled matmul) so every DVE operand of the
                    # evacuation shares its base partition with the ctxT
                    # destination rows.
                    cx0 = ab_pool.tile([P, NT], F32, tag="cx0")
                    cx1 = ab_pool.tile([P, NT], F32, tag="cx1")
                    sm0 = ab_pool.tile([P, NT], F32, tag="sm0")
                    sm1 = ab_pool.tile([P, NT], F32, tag="sm1")
                    for sb in range(NB):
                        s0 = sb * P
                        sc = sc_pool.tile([P, 2 * NT], F32, tag="sc")
                        # scT[s, t] = sum_d KT[d, s] QT[d, t], head pair
                        # row-tiled on the PE (K=64 at rows 0 / 64).
                        nc.tensor.matmul(
                            sc[:, 0:NT],
                            kt[0:DH, hp, s0:s0 + P],
                            qt[0:DH, hp, t0:t0 + NT])
                        nc.tensor.matmul(
                            sc[:, NT:2 * NT],
                            kt[DH:P, hp, s0:s0 + P],
                            qt[DH:P, hp, t0:t0 + NT])
                        # attnT = exp(scale * scT + mask[s])  (bf16)
                        attn = attn_pool.tile([P, 2 * NT], BF16, tag="attn")
                        nc.scalar.activation(
                            attn[:], sc[:], mybir.ActivationFunctionType.Exp,
                            bias=mask_t[:, sb:sb + 1], scale=SCALE)
                        # ctxT[d, t] += V[s, d]^T attnT[s, t]; sums rows are
                        # pre-broadcast over 64 partitions by the all-ones
                        # stationary.
                        st = (sb == 0)
                        sp = (sb == NB - 1)
                        nc.tensor.matmul(cx0[0:DH, :],
                                         v_sb[:, sb, h0 * DH:(h0 + 1) * DH],
                                         attn[:, 0:NT], start=st, stop=sp)
                        nc.tensor.matmul(sm0[0:DH, :], ones64[:],
                                         attn[:, 0:NT], start=st, stop=sp)
                        nc.tensor.matmul(cx1[DH:P, :],
                                         v_sb[:, sb, h1 * DH:(h1 + 1) * DH],
                                         attn[:, NT:2 * NT], start=st, stop=sp)
                        nc.tensor.matmul(sm1[DH:P, :], ones64[:],
                                         attn[:, NT:2 * NT], start=st, stop=sp)
                    # normalize + evacuate: ctxT = ctx * (1 / sums)
                    r = recip_pool.tile([P, NT], F32, tag="r")
                    nc.vector.reciprocal(r[0:DH, :], sm0[0:DH, :])
                    nc.vector.reciprocal(r[DH:P, :], sm1[DH:P, :])
                    nc.vector.tensor_mul(ctxT[0:DH, hp, t0:t0 + NT],
                                         cx0[0:DH, :], r[0:DH, :])
                    nc.vector.tensor_mul(ctxT[DH:P, hp, t0:t0 + NT],
                                         cx1[DH:P, :], r[DH:P, :])

        # ---- output projection: out[t, e''] = ctxT^T @ w_out + b_out ----
        with tc.tile_pool(name="out_psum", bufs=2, space="PSUM") as out_psum:
            for m in range(NB):
                po = out_psum.tile([P, E], F32, tag="po")
                for c in range(E // NT):
                    n0 = c * NT
                    for k in range(NB):
                        nc.tensor.matmul(
                            po[:, n0:n0 + NT],
                            ctxT[:, k, m * P:(m + 1) * P],
                            wo_bf[:, k, n0:n0 + NT],
                            start=(k == 0), stop=False)
                    nc.tensor.matmul(po[:, n0:n0 + NT], ones_k1[:],
                                     bo_sb[:, n0:n0 + NT], start=False,
                                     stop=True)
                os_t = stage.tile([P, E], F32, tag="os")
                nc.scalar.copy(os_t[:], po[:])
                nc.sync.dma_start(out_d.ap()[m * P:(m + 1) * P, :], os_t[:])


_NC_CACHE = {}


def _get_nc(reps: int = 1):
    if reps not in _NC_CACHE:
        _NC_CACHE[reps] = build_kernel(reps)
    return _NC_CACHE[reps]


def make_in_maps(x, mask, w_q, w_k, w_v, w_out, b_out):
    x = np.asarray(x, dtype=np.float32)
    mask = np.asarray(mask, dtype=np.float32)
    common = {
        "w_q": np.ascontiguousarray(w_q, dtype=np.float32),
        "w_k": np.ascontiguousarray(w_k, dtype=np.float32),
        "w_v": np.ascontiguousarray(w_v, dtype=np.float32),
        "w_out": np.ascontiguousarray(w_out, dtype=np.float32),
        "b_out": np.ascontiguousarray(b_out, dtype=np.float32),
    }
    in_maps = []
    for c in range(N_CORES):
        m = dict(common)
        m["x"] = np.ascontiguousarray(x[:, c, :])
        m["mask"] = np.ascontiguousarray(mask[c])
        in_maps.append(m)
    return in_maps


def kernel(x, mask, w_q, w_k, w_v, w_out, b_out):
    nc = _get_nc(1)
    in_maps = make_in_maps(x, mask, w_q, w_k, w_v, w_out, b_out)
    res = run_bass_kernel_spmd(nc, in_maps, list(range(N_CORES)))
    out = np.stack([res.results[c]["out"] for c in range(N_CORES)], axis=1)
    return out.astype(np.float32)
